# revision 65
# baseline (speedup 1.0000x reference)
"""DilateAttention Trainium2 kernel (nn_DilateAttention).

Full inputs q,k,v: [8, 192, 56, 56] fp32. Output: [8, 56, 56, 192] fp32.
Sharded data-parallel over batch B=8 across 8 NeuronCores.

Per-core layout: channels-on-partitions. Head group G0 (heads 0-3) fills 128
partitions directly. Group G1 (heads 4-5, 64 channels) is PIXEL-SPLIT: the
image's two halves (28 rows each, with halo) are stacked on partitions
0-63 / 64-127, so every vector op runs at full 128-lane width.

Dilated-window shifts are pure free-dim AP offsets into zero-padded k/v
images. Cross-partition reductions (sum over head_dim) and broadcasts (attn
weights over head_dim) run on the PE via 0/1 selector matmuls in float32r
(full rate, ~1.2e-4 rounding); exp on ScalarE; elementwise mul/add split
across VectorE and GpSimd.
"""

import sys

for _p in ("/opt/trn_rl_repo",):
    if _p not in sys.path:
        sys.path.insert(0, _p)

import numpy as np

B = 8
C = 192
H = W = 56
HD = 32
NH = 6  # heads
KK = 9  # kernel*kernel
SCALE = HD ** -0.5
HWPIX = H * W  # 3136
HALF = HWPIX // 2  # 1568
HROWS = H // 2  # 28
SHIFTS = [(di, dj) for di in (-2, 0, 2) for dj in (-2, 0, 2)]
NROWS = KK * 12  # 108 score rows, row m = j*12 + h*2 + half

# G0 padded image geometry: rows y in [-2,58), cols x in [-2,58)
PADH = PADW = 60
ROW0 = COL0 = 2
# G1 dup geometry: [128, 32, 60]; lower p<64: y in [-2,30); upper: y in [26,58)
PADH1 = 32

USE_FP32R = True


def _build_consts():
    """Selector constants for the [108, 1568] score layout.

    Score row m = j*12 + h*2 + half  (j in [0,9), h in [0,6), half in {0,1}).
    """
    consts = {}
    NR = 12 * KK  # 108
    # selA0w: [128, 9, 109]; window [:, j, 1:109] = half0, [:, j, 0:108] = half1
    a = np.zeros((128, KK, NR + 1), np.float32)
    for p in range(128):
        for j in range(KK):
            a[p, j, j * 12 + (p // HD) * 2 + 1] = 1.0
    consts["selA0w"] = a.reshape(128, KK * (NR + 1))
    # selA1: [128, 9, 108] for the G1 dup prod (half encoded in partition)
    a = np.zeros((128, KK, NR), np.float32)
    for p in range(128):
        hh = (4 + (p % 64) // HD) * 2 + p // 64
        for j in range(KK):
            a[p, j, j * 12 + hh] = 1.0
    consts["selA1"] = a.reshape(128, KK * NR)
    # selB0lo/hi: [108, 9, 128] lhsT for G0 attn broadcast
    for half in (0, 1):
        b = np.zeros((NR, KK, 128), np.float32)
        for j in range(KK):
            for p in range(128):
                b[j * 12 + (p // HD) * 2 + half, j, p] = 1.0
        consts[f"selB0h{half}"] = b.reshape(NR, KK * 128)
    # selB1: [108, 9, 128] attn broadcast for G1 dup (half from partition)
    b = np.zeros((NR, KK, 128), np.float32)
    for j in range(KK):
        for p in range(128):
            b[j * 12 + (4 + (p % 64) // HD) * 2 + p // 64, j, p] = 1.0
    consts["selB1"] = b.reshape(NR, KK * 128)
    # selD: [108, 12] sum over j per (head, half)
    d = np.zeros((NR, 12), np.float32)
    for m in range(NR):
        d[m, m % 12] = 1.0
    consts["selD"] = d
    # ident: [128, 128] identity for PE pass-through accumulation
    consts["ident"] = np.eye(128, dtype=np.float32)
    # selN: [12, 108] broadcast per-(head,half) value to all j rows
    n = np.zeros((12, NR), np.float32)
    for m in range(NR):
        n[m % 12, m] = 1.0
    consts["selN"] = n
    return consts


def _bank_chunks(c0, c1):
    """Split [c0,c1) at 512-element PSUM bank boundaries."""
    out = []
    while c0 < c1:
        nxt = min((c0 // 512 + 1) * 512, c1)
        out.append((c0, nxt))
        c0 = nxt
    return out


def build_module():
    import concourse.bacc as bacc
    import concourse.mybir as mybir
    import concourse.tile as tile

    fp32 = mybir.dt.float32
    mmdt = mybir.dt.float32r if USE_FP32R else fp32
    AL = mybir.AluOpType

    nc = bacc.Bacc("TRN2", target_bir_lowering=False, debug=False, num_devices=B)

    q_d = nc.dram_tensor("q", [C, H, W], fp32, kind="ExternalInput")
    k_d = nc.dram_tensor("k", [C, H, W], fp32, kind="ExternalInput")
    v_d = nc.dram_tensor("v", [C, H, W], fp32, kind="ExternalInput")
    o_d = nc.dram_tensor("o", [HWPIX, C], fp32, kind="ExternalOutput")
    consts = _build_consts()
    c_d = {
        name: nc.dram_tensor(name, list(arr.shape), mmdt, kind="ExternalInput")
        for name, arr in consts.items()
    }

    with tile.TileContext(nc) as tc:
        with (
            tc.tile_pool(name="io", bufs=2) as io_pool,
            tc.tile_pool(name="work", bufs=2) as work_pool,
            tc.tile_pool(name="tree", bufs=3) as tree_pool,
            tc.tile_pool(name="small", bufs=1) as small_pool,
        ):
            def load_g1_dup(dst_name, src_d, eng, eng2=None):
                """[128, 32, 60] dup tile: lower y in [-2,30), upper y in [26,58)."""
                t = io_pool.tile([128, PADH1, PADW], fp32, tag="kv", name=dst_name)
                nc.gpsimd.memset(t[0:64, 0:ROW0, :], 0.0)
                nc.gpsimd.memset(t[64:128, 30:32, :], 0.0)
                nc.gpsimd.memset(t[:, :, 0:COL0], 0.0)
                nc.gpsimd.memset(t[:, :, COL0 + W :], 0.0)
                eng.dma_start(t[0:64, ROW0 : ROW0 + 30, COL0 : COL0 + W], src_d[128:192, 0:30, :])
                (eng2 or eng).dma_start(t[64:128, 0:30, COL0 : COL0 + W], src_d[128:192, 26:56, :])
                return t

            def load_g1_q(eng, eng2=None):
                t = io_pool.tile([128, HROWS, W], fp32, tag="q", name="q1")
                eng.dma_start(t[0:64, :, :], q_d[128:192, 0:HROWS, :])
                (eng2 or eng).dma_start(t[64:128, :, :], q_d[128:192, HROWS:H, :])
                return t

            def load_g0_pad(dst_name, src_d, eng, eng2=None):
                t = io_pool.tile([128, PADH, PADW], fp32, tag="kv", name=dst_name)
                nc.gpsimd.memset(t[:, 0:ROW0, :], 0.0)
                nc.gpsimd.memset(t[:, ROW0 + H :, :], 0.0)
                nc.gpsimd.memset(t[:, ROW0 : ROW0 + H, 0:COL0], 0.0)
                nc.gpsimd.memset(t[:, ROW0 : ROW0 + H, COL0 + W :], 0.0)
                if eng2 is None:
                    eng.dma_start(t[:, ROW0 : ROW0 + H, COL0 : COL0 + W], src_d[0:128, :, :])
                else:
                    eng.dma_start(
                        t[:, ROW0 : ROW0 + HROWS, COL0 : COL0 + W], src_d[0:128, 0:HROWS, :]
                    )
                    eng2.dma_start(
                        t[:, ROW0 + HROWS : ROW0 + H, COL0 : COL0 + W],
                        src_d[0:128, HROWS:H, :],
                    )
                return t

            # G1 first (smaller: compute starts sooner); constants interleaved
            # by need-time: selA1 right after q1, the rest after k0/q0
            sel_sb = {}

            def load_const(name, eng):
                arr = consts[name]
                t = small_pool.tile(list(arr.shape), mmdt, tag=f"c_{name}", name=f"c_{name}")
                eng.dma_start(t[:], c_d[name][:])
                sel_sb[name] = t

            with tc.high_priority():
                load_const("selA1", nc.scalar)
                k1 = load_g1_dup("k1", k_d, nc.sync, nc.scalar)
                q1 = load_g1_q(nc.scalar, nc.sync)

            E_sb = small_pool.tile([NROWS, HALF], mmdt, tag="E")

            # ---- stage A: scores. S_ps[m=(j*12+h*2+half), px] = sum_d q*k_shift
            with tc.tile_pool(name="psS", bufs=1, space="PSUM") as psS_pool:
                S_ps = psS_pool.tile([NROWS, HALF], fp32, tag="S")
                selA1 = sel_sb["selA1"].rearrange("p (j m) -> p j m", j=KK)
                # G1 (dup): 9 muls [128, 28, 56]; one MM pass (half in rows)
                for j, (di, dj) in enumerate(SHIFTS):
                    prod = work_pool.tile([128, HROWS, W], mmdt, tag="prod", bufs=6, name="prod1")
                    kv = k1[:, ROW0 + di : ROW0 + di + HROWS, COL0 + dj : COL0 + dj + W]
                    a_eng = nc.gpsimd if j in (3, 6) else nc.vector
                    a_eng.tensor_tensor(prod[:], q1[:], kv, AL.mult)
                    pflat = prod.rearrange("p a b -> p (a b)")
                    for n0, n1 in _bank_chunks(0, HALF):
                        nc.tensor.matmul(
                            S_ps[:, n0:n1],
                            selA1[:, j, :],
                            pflat[:, n0:n1],
                            start=(j == 0),
                            stop=False,
                        )
                    if j == 0:
                        q0 = io_pool.tile([128, H, W], fp32, tag="q", name="q0")
                        nc.scalar.dma_start(q0[:], q_d[0:128, :, :])
                        k0 = load_g0_pad("k0", k_d, nc.sync, nc.scalar)
                    elif j == 2:
                        load_const("selA0w", nc.scalar)
                        load_const("selD", nc.sync)
                        load_const("selN", nc.sync)
                    elif j == 5:
                        load_const("selB0h0", nc.scalar)
                        load_const("selB0h1", nc.sync)
                        load_const("selB1", nc.sync)
                        load_const("ident", nc.sync)
                # G0: 9 muls [128, 56, 56]; two MM passes (one per pixel half)
                selA0w = sel_sb["selA0w"].rearrange("p (j m) -> p j m", j=KK)
                selA0h = [selA0w[:, :, 1 : NROWS + 1], selA0w[:, :, 0:NROWS]]
                for j, (di, dj) in enumerate(SHIFTS):
                    prod = work_pool.tile([128, H, W], mmdt, tag="prod", bufs=6, name="prod0")
                    kv = k0[:, ROW0 + di : ROW0 + di + H, COL0 + dj : COL0 + dj + W]
                    a_eng = nc.gpsimd if j in (2, 5) else nc.vector
                    a_eng.tensor_tensor(prod[:], q0[:], kv, AL.mult)
                    pflat = prod.rearrange("p a b -> p (a b)")
                    for half in (0, 1):
                        for n0, n1 in _bank_chunks(0, HALF):
                            nc.tensor.matmul(
                                S_ps[:, n0:n1],
                                selA0h[half][:, j, :],
                                pflat[:, half * HALF + n0 : half * HALF + n1],
                                start=False,
                                stop=(j == KK - 1 and half == 1),
                            )

                # exp(scale * S), evacuating PSUM (split so B starts earlier)
                for e0, e1 in ((0, HALF // 2), (HALF // 2, HALF)):
                    nc.scalar.activation(
                        E_sb[:, e0:e1],
                        S_ps[:, e0:e1],
                        mybir.ActivationFunctionType.Exp,
                        scale=float(SCALE),
                    )

            # ---- stage B: normalize E by sum over j (chunked through PSUM)
            CHB = 784  # 2 chunks of 784 = 1568
            with tc.tile_pool(name="psB", bufs=2, space="PSUM") as psB_pool:
                for n0 in range(0, HALF, CHB):
                    n1 = n0 + CHB
                    D_ps = psB_pool.tile([12, CHB], fp32, tag="D")
                    for c0, c1 in _bank_chunks(0, CHB):
                        nc.tensor.matmul(
                            D_ps[:, c0:c1],
                            sel_sb["selD"][:],
                            E_sb[:, n0 + c0 : n0 + c1],
                            start=True,
                            stop=True,
                        )
                    R_ch = small_pool.tile([12, CHB], fp32, tag="R", bufs=2)
                    nc.vector.reciprocal_approx_fast(R_ch[:], D_ps[:])
                    Rr_ch = small_pool.tile([12, CHB], mmdt, tag="Rr", bufs=2)
                    nc.vector.tensor_copy(Rr_ch[:], R_ch[:])
                    RB_ps = psB_pool.tile([NROWS, CHB], fp32, tag="RB")
                    for c0, c1 in _bank_chunks(0, CHB):
                        nc.tensor.matmul(
                            RB_ps[:, c0:c1],
                            sel_sb["selN"][:],
                            Rr_ch[:, c0:c1],
                            start=True,
                            stop=True,
                        )
                    nc.vector.tensor_tensor(E_sb[:, n0:n1], E_sb[:, n0:n1], RB_ps[:], AL.mult)

            # ---- load padded v (reuses k slots)
            v0 = load_g0_pad("v0", v_d, nc.sync, nc.scalar)
            v1 = load_g1_dup("v1", v_d, nc.scalar)

            # ---- stage C + output, per group
            selB0h = [
                sel_sb["selB0h0"].rearrange("m (j p) -> m j p", j=KK),
                sel_sb["selB0h1"].rearrange("m (j p) -> m j p", j=KK),
            ]
            selB1 = sel_sb["selB1"].rearrange("m (j p) -> m j p", j=KK)

            def do_group(g, psC_pool, ident):
                npx = HALF if g == 1 else HWPIX
                nhalves = 1 if g == 1 else 2
                o_view = o_d.ap().rearrange("(bp pi) c -> pi bp c", pi=32)

                for hf in range(nhalves):
                    ACC_ps = psC_pool.tile([128, HALF], fp32, tag="ACC", name=f"ACC{g}{hf}")
                    started, stopped = set(), set()
                    # last-touch map for stop flags
                    seg_all = []
                    for qi in (0, 1):
                        for n0, n1 in _bank_chunks(qi * (HALF // 2), (qi + 1) * (HALF // 2)):
                            seg_all.append((qi, n0, n1))
                    last_by_bank = {}
                    for qi, n0, n1 in seg_all:
                        last_by_bank[n0 // 512] = (qi, n0)
                    for j, (di, dj) in enumerate(SHIFTS):
                        for qi in (0, 1):  # row-quarters of 14 rows = 784 px
                            sel = selB1 if g == 1 else selB0h[hf]
                            vt = v1 if g == 1 else v0
                            ab_ps = psC_pool.tile([128, HALF // 2], fp32, tag="AB", bufs=2)
                            for n0, n1 in _bank_chunks(0, HALF // 2):
                                e0 = qi * (HALF // 2) + n0
                                nc.tensor.matmul(
                                    ab_ps[:, n0:n1], sel[:, j, :],
                                    E_sb[:, e0 : e0 + (n1 - n0)], start=True, stop=True,
                                )
                            r0 = ROW0 + di + (hf * HROWS if g == 0 else 0) + qi * (HROWS // 2)
                            vv = vt[:, r0 : r0 + HROWS // 2, COL0 + dj : COL0 + dj + W]
                            prod = tree_pool.tile(
                                [128, HROWS // 2, W], mmdt, tag="prod", bufs=4, name=f"cprod{g}"
                            )
                            nc.vector.tensor_tensor(
                                prod[:],
                                ab_ps.rearrange("p (a b) -> p a b", a=HROWS // 2),
                                vv,
                                AL.mult,
                            )
                            pf = prod.rearrange("p a b -> p (a b)")
                            # PE identity-accumulate into ACC
                            for n0, n1 in _bank_chunks(qi * (HALF // 2), (qi + 1) * (HALF // 2)):
                                bank = n0 // 512
                                st = j == 0 and bank not in started
                                if st:
                                    started.add(bank)
                                sp = j == KK - 1 and last_by_bank[bank] == (qi, n0)
                                nc.tensor.matmul(
                                    ACC_ps[:, n0:n1],
                                    ident[:],
                                    pf[:, n0 - qi * (HALF // 2) : n1 - qi * (HALF // 2)],
                                    start=st,
                                    stop=sp,
                                )
                    # transpose (PSUM -> SBUF) + output DMA for this half
                    t_sb = tree_pool.tile([128, HALF], fp32, tag="tout", bufs=2, name=f"t{g}{hf}")
                    nc.vector.transpose(t_sb[:], ACC_ps[:])
                    for bc in range(4):
                        src_ap = t_sb[bc * 32 : (bc + 1) * 32, :].rearrange(
                            "p (bp ci) -> p bp ci", ci=32
                        )
                        if g == 1:
                            c0 = 128 + (bc % 2) * 32
                            pxoff = (bc // 2) * (HALF // 32)
                            dst = o_view[:, pxoff : pxoff + HALF // 32, c0 : c0 + 32]
                        else:
                            c0 = bc * 32
                            pxoff = hf * (HALF // 32)
                            dst = o_view[:, pxoff : pxoff + HALF // 32, c0 : c0 + 32]
                        (nc.sync if bc % 2 == 0 else nc.scalar).dma_start(dst, src_ap)

            with tc.tile_pool(name="psC", bufs=1, space="PSUM") as psC_pool:
                ident = sel_sb["ident"]
                do_group(0, psC_pool, ident)
                do_group(1, psC_pool, ident)

    nc.compile()
    return nc, consts


_CACHE = {}


def _get_module():
    if "nc" not in _CACHE:
        _CACHE["nc"], _CACHE["consts"] = build_module()
    return _CACHE["nc"], _CACHE["consts"]


def make_in_maps(q, k, v, consts):
    in_maps = []
    for b in range(B):
        m = {
            "q": np.ascontiguousarray(q[b].reshape(C, H, W)),
            "k": np.ascontiguousarray(k[b].reshape(C, H, W)),
            "v": np.ascontiguousarray(v[b].reshape(C, H, W)),
        }
        m.update(consts)
        in_maps.append(m)
    return in_maps


def kernel(q: np.ndarray, k: np.ndarray, v: np.ndarray) -> np.ndarray:
    from concourse import bass_utils

    nc, consts = _get_module()
    in_maps = make_in_maps(np.asarray(q), np.asarray(k), np.asarray(v), consts)
    res = bass_utils.run_bass_kernel_spmd(nc, in_maps, core_ids=list(range(B)))
    out = np.stack([r["o"].reshape(H, W, C) for r in res.results])
    return out


# revision 66
# speedup vs baseline: 1.0020x; 1.0020x over previous
"""DilateAttention Trainium2 kernel (nn_DilateAttention).

Full inputs q,k,v: [8, 192, 56, 56] fp32. Output: [8, 56, 56, 192] fp32.
Sharded data-parallel over batch B=8 across 8 NeuronCores.

Per-core layout: channels-on-partitions. Head group G0 (heads 0-3) fills 128
partitions directly. Group G1 (heads 4-5, 64 channels) is PIXEL-SPLIT: the
image's two halves (28 rows each, with halo) are stacked on partitions
0-63 / 64-127, so every vector op runs at full 128-lane width.

Dilated-window shifts are pure free-dim AP offsets into zero-padded k/v
images. Cross-partition reductions (sum over head_dim) and broadcasts (attn
weights over head_dim) run on the PE via 0/1 selector matmuls in float32r
(full rate, ~1.2e-4 rounding); exp on ScalarE; elementwise mul/add split
across VectorE and GpSimd.
"""

import sys

for _p in ("/opt/trn_rl_repo",):
    if _p not in sys.path:
        sys.path.insert(0, _p)

import numpy as np

B = 8
C = 192
H = W = 56
HD = 32
NH = 6  # heads
KK = 9  # kernel*kernel
SCALE = HD ** -0.5
HWPIX = H * W  # 3136
HALF = HWPIX // 2  # 1568
HROWS = H // 2  # 28
SHIFTS = [(di, dj) for di in (-2, 0, 2) for dj in (-2, 0, 2)]
NROWS = KK * 12  # 108 score rows, row m = j*12 + h*2 + half

# G0 padded image geometry: rows y in [-2,58), cols x in [-2,58)
PADH = PADW = 60
ROW0 = COL0 = 2
# G1 dup geometry: [128, 32, 60]; lower p<64: y in [-2,30); upper: y in [26,58)
PADH1 = 32

USE_FP32R = True


def _build_consts():
    """Selector constants for the [108, 1568] score layout.

    Score row m = j*12 + h*2 + half  (j in [0,9), h in [0,6), half in {0,1}).
    """
    consts = {}
    NR = 12 * KK  # 108
    # selA0w: [128, 9, 109]; window [:, j, 1:109] = half0, [:, j, 0:108] = half1
    a = np.zeros((128, KK, NR + 1), np.float32)
    for p in range(128):
        for j in range(KK):
            a[p, j, j * 12 + (p // HD) * 2 + 1] = 1.0
    consts["selA0w"] = a.reshape(128, KK * (NR + 1))
    # selA1: [128, 9, 108] for the G1 dup prod (half encoded in partition)
    a = np.zeros((128, KK, NR), np.float32)
    for p in range(128):
        hh = (4 + (p % 64) // HD) * 2 + p // 64
        for j in range(KK):
            a[p, j, j * 12 + hh] = 1.0
    consts["selA1"] = a.reshape(128, KK * NR)
    # selB0lo/hi: [108, 9, 128] lhsT for G0 attn broadcast
    for half in (0, 1):
        b = np.zeros((NR, KK, 128), np.float32)
        for j in range(KK):
            for p in range(128):
                b[j * 12 + (p // HD) * 2 + half, j, p] = 1.0
        consts[f"selB0h{half}"] = b.reshape(NR, KK * 128)
    # selB1: [108, 9, 128] attn broadcast for G1 dup (half from partition)
    b = np.zeros((NR, KK, 128), np.float32)
    for j in range(KK):
        for p in range(128):
            b[j * 12 + (4 + (p % 64) // HD) * 2 + p // 64, j, p] = 1.0
    consts["selB1"] = b.reshape(NR, KK * 128)
    # selD: [108, 12] sum over j per (head, half)
    d = np.zeros((NR, 12), np.float32)
    for m in range(NR):
        d[m, m % 12] = 1.0
    consts["selD"] = d
    # ident: [128, 128] identity for PE pass-through accumulation
    consts["ident"] = np.eye(128, dtype=np.float32)
    # selN: [12, 108] broadcast per-(head,half) value to all j rows
    n = np.zeros((12, NR), np.float32)
    for m in range(NR):
        n[m % 12, m] = 1.0
    consts["selN"] = n
    return consts


def _bank_chunks(c0, c1):
    """Split [c0,c1) at 512-element PSUM bank boundaries."""
    out = []
    while c0 < c1:
        nxt = min((c0 // 512 + 1) * 512, c1)
        out.append((c0, nxt))
        c0 = nxt
    return out


def build_module():
    import concourse.bacc as bacc
    import concourse.mybir as mybir
    import concourse.tile as tile

    fp32 = mybir.dt.float32
    mmdt = mybir.dt.float32r if USE_FP32R else fp32
    AL = mybir.AluOpType

    nc = bacc.Bacc("TRN2", target_bir_lowering=False, debug=False, num_devices=B)

    q_d = nc.dram_tensor("q", [C, H, W], fp32, kind="ExternalInput")
    k_d = nc.dram_tensor("k", [C, H, W], fp32, kind="ExternalInput")
    v_d = nc.dram_tensor("v", [C, H, W], fp32, kind="ExternalInput")
    o_d = nc.dram_tensor("o", [HWPIX, C], fp32, kind="ExternalOutput")
    consts = _build_consts()
    c_d = {
        name: nc.dram_tensor(name, list(arr.shape), mmdt, kind="ExternalInput")
        for name, arr in consts.items()
    }

    with tile.TileContext(nc) as tc:
        with (
            tc.tile_pool(name="io", bufs=2) as io_pool,
            tc.tile_pool(name="work", bufs=2) as work_pool,
            tc.tile_pool(name="tree", bufs=3) as tree_pool,
            tc.tile_pool(name="small", bufs=1) as small_pool,
        ):
            def load_g1_dup(dst_name, src_d, eng, eng2=None):
                """[128, 32, 60] dup tile: lower y in [-2,30), upper y in [26,58)."""
                t = io_pool.tile([128, PADH1, PADW], fp32, tag="kv", name=dst_name)
                nc.gpsimd.memset(t[0:64, 0:ROW0, :], 0.0)
                nc.gpsimd.memset(t[64:128, 30:32, :], 0.0)
                nc.gpsimd.memset(t[:, :, 0:COL0], 0.0)
                nc.gpsimd.memset(t[:, :, COL0 + W :], 0.0)
                eng.dma_start(t[0:64, ROW0 : ROW0 + 30, COL0 : COL0 + W], src_d[128:192, 0:30, :])
                (eng2 or eng).dma_start(t[64:128, 0:30, COL0 : COL0 + W], src_d[128:192, 26:56, :])
                return t

            def load_g1_q(eng, eng2=None):
                t = io_pool.tile([128, HROWS, W], fp32, tag="q", name="q1")
                eng.dma_start(t[0:64, :, :], q_d[128:192, 0:HROWS, :])
                (eng2 or eng).dma_start(t[64:128, :, :], q_d[128:192, HROWS:H, :])
                return t

            def load_g0_pad(dst_name, src_d, eng, eng2=None):
                t = io_pool.tile([128, PADH, PADW], fp32, tag="kv", name=dst_name)
                nc.gpsimd.memset(t[:, 0:ROW0, :], 0.0)
                nc.gpsimd.memset(t[:, ROW0 + H :, :], 0.0)
                nc.gpsimd.memset(t[:, ROW0 : ROW0 + H, 0:COL0], 0.0)
                nc.gpsimd.memset(t[:, ROW0 : ROW0 + H, COL0 + W :], 0.0)
                if eng2 is None:
                    eng.dma_start(t[:, ROW0 : ROW0 + H, COL0 : COL0 + W], src_d[0:128, :, :])
                else:
                    eng.dma_start(
                        t[:, ROW0 : ROW0 + HROWS, COL0 : COL0 + W], src_d[0:128, 0:HROWS, :]
                    )
                    eng2.dma_start(
                        t[:, ROW0 + HROWS : ROW0 + H, COL0 : COL0 + W],
                        src_d[0:128, HROWS:H, :],
                    )
                return t

            # G1 first (smaller: compute starts sooner); constants interleaved
            # by need-time: selA1 right after q1, the rest after k0/q0
            sel_sb = {}

            def load_const(name, eng):
                arr = consts[name]
                t = small_pool.tile(list(arr.shape), mmdt, tag=f"c_{name}", name=f"c_{name}")
                eng.dma_start(t[:], c_d[name][:])
                sel_sb[name] = t

            with tc.high_priority():
                load_const("selA1", nc.scalar)
                k1 = load_g1_dup("k1", k_d, nc.sync, nc.scalar)
                q1 = load_g1_q(nc.scalar, nc.sync)

            E_sb = small_pool.tile([NROWS, HALF], mmdt, tag="E")

            # ---- stage A: scores. S_ps[m=(j*12+h*2+half), px] = sum_d q*k_shift
            with tc.tile_pool(name="psS", bufs=1, space="PSUM") as psS_pool:
                S_ps = psS_pool.tile([NROWS, HALF], fp32, tag="S")
                selA1 = sel_sb["selA1"].rearrange("p (j m) -> p j m", j=KK)
                # G1 (dup): 9 muls [128, 28, 56]; one MM pass (half in rows)
                for j, (di, dj) in enumerate(SHIFTS):
                    prod = work_pool.tile([128, HROWS, W], mmdt, tag="prod", bufs=6, name="prod1")
                    kv = k1[:, ROW0 + di : ROW0 + di + HROWS, COL0 + dj : COL0 + dj + W]
                    a_eng = nc.gpsimd if j in (3, 6) else nc.vector
                    a_eng.tensor_tensor(prod[:], q1[:], kv, AL.mult)
                    pflat = prod.rearrange("p a b -> p (a b)")
                    for n0, n1 in _bank_chunks(0, HALF):
                        nc.tensor.matmul(
                            S_ps[:, n0:n1],
                            selA1[:, j, :],
                            pflat[:, n0:n1],
                            start=(j == 0),
                            stop=False,
                        )
                    if j == 0:
                        q0 = io_pool.tile([128, H, W], fp32, tag="q", name="q0")
                        nc.scalar.dma_start(q0[:], q_d[0:128, :, :])
                        k0 = load_g0_pad("k0", k_d, nc.sync, nc.scalar)
                    elif j == 2:
                        load_const("selA0w", nc.scalar)
                        load_const("selD", nc.sync)
                        load_const("selN", nc.sync)
                    elif j == 5:
                        load_const("selB0h0", nc.scalar)
                        load_const("selB0h1", nc.sync)
                        load_const("selB1", nc.sync)
                        load_const("ident", nc.sync)
                # G0: 9 muls [128, 56, 56]; two MM passes (one per pixel half)
                selA0w = sel_sb["selA0w"].rearrange("p (j m) -> p j m", j=KK)
                selA0h = [selA0w[:, :, 1 : NROWS + 1], selA0w[:, :, 0:NROWS]]
                for j, (di, dj) in enumerate(SHIFTS):
                    for half in (0, 1):
                        prod = work_pool.tile(
                            [128, HROWS, W], mmdt, tag="prod", bufs=6, name="prod0"
                        )
                        kv = k0[
                            :,
                            ROW0 + di + half * HROWS : ROW0 + di + (half + 1) * HROWS,
                            COL0 + dj : COL0 + dj + W,
                        ]
                        qv = q0[:, half * HROWS : (half + 1) * HROWS, :]
                        a_eng = nc.gpsimd if j in (2, 5) else nc.vector
                        a_eng.tensor_tensor(prod[:], qv, kv, AL.mult)
                        pflat = prod.rearrange("p a b -> p (a b)")
                        for n0, n1 in _bank_chunks(0, HALF):
                            nc.tensor.matmul(
                                S_ps[:, n0:n1],
                                selA0h[half][:, j, :],
                                pflat[:, n0:n1],
                                start=False,
                                stop=(j == KK - 1 and half == 1),
                            )

                # exp(scale * S), evacuating PSUM (split so B starts earlier)
                for e0, e1 in ((0, HALF // 2), (HALF // 2, HALF)):
                    nc.scalar.activation(
                        E_sb[:, e0:e1],
                        S_ps[:, e0:e1],
                        mybir.ActivationFunctionType.Exp,
                        scale=float(SCALE),
                    )

            # ---- stage B: normalize E by sum over j (chunked through PSUM)
            CHB = 784  # 2 chunks of 784 = 1568
            with tc.tile_pool(name="psB", bufs=2, space="PSUM") as psB_pool:
                for n0 in range(0, HALF, CHB):
                    n1 = n0 + CHB
                    D_ps = psB_pool.tile([12, CHB], fp32, tag="D")
                    for c0, c1 in _bank_chunks(0, CHB):
                        nc.tensor.matmul(
                            D_ps[:, c0:c1],
                            sel_sb["selD"][:],
                            E_sb[:, n0 + c0 : n0 + c1],
                            start=True,
                            stop=True,
                        )
                    R_ch = small_pool.tile([12, CHB], fp32, tag="R", bufs=2)
                    nc.vector.reciprocal_approx_fast(R_ch[:], D_ps[:])
                    Rr_ch = small_pool.tile([12, CHB], mmdt, tag="Rr", bufs=2)
                    nc.vector.tensor_copy(Rr_ch[:], R_ch[:])
                    RB_ps = psB_pool.tile([NROWS, CHB], fp32, tag="RB")
                    for c0, c1 in _bank_chunks(0, CHB):
                        nc.tensor.matmul(
                            RB_ps[:, c0:c1],
                            sel_sb["selN"][:],
                            Rr_ch[:, c0:c1],
                            start=True,
                            stop=True,
                        )
                    nc.vector.tensor_tensor(E_sb[:, n0:n1], E_sb[:, n0:n1], RB_ps[:], AL.mult)

            # ---- load padded v (reuses k slots)
            v0 = load_g0_pad("v0", v_d, nc.sync, nc.scalar)
            v1 = load_g1_dup("v1", v_d, nc.scalar)

            # ---- stage C + output, per group
            selB0h = [
                sel_sb["selB0h0"].rearrange("m (j p) -> m j p", j=KK),
                sel_sb["selB0h1"].rearrange("m (j p) -> m j p", j=KK),
            ]
            selB1 = sel_sb["selB1"].rearrange("m (j p) -> m j p", j=KK)

            def do_group(g, psC_pool, ident):
                npx = HALF if g == 1 else HWPIX
                nhalves = 1 if g == 1 else 2
                o_view = o_d.ap().rearrange("(bp pi) c -> pi bp c", pi=32)

                for hf in range(nhalves):
                    ACC_ps = psC_pool.tile([128, HALF], fp32, tag="ACC", name=f"ACC{g}{hf}")
                    started, stopped = set(), set()
                    # last-touch map for stop flags
                    seg_all = []
                    for qi in (0, 1):
                        for n0, n1 in _bank_chunks(qi * (HALF // 2), (qi + 1) * (HALF // 2)):
                            seg_all.append((qi, n0, n1))
                    last_by_bank = {}
                    for qi, n0, n1 in seg_all:
                        last_by_bank[n0 // 512] = (qi, n0)
                    for j, (di, dj) in enumerate(SHIFTS):
                        for qi in (0, 1):  # row-quarters of 14 rows = 784 px
                            sel = selB1 if g == 1 else selB0h[hf]
                            vt = v1 if g == 1 else v0
                            ab_ps = psC_pool.tile([128, HALF // 2], fp32, tag="AB", bufs=2)
                            for n0, n1 in _bank_chunks(0, HALF // 2):
                                e0 = qi * (HALF // 2) + n0
                                nc.tensor.matmul(
                                    ab_ps[:, n0:n1], sel[:, j, :],
                                    E_sb[:, e0 : e0 + (n1 - n0)], start=True, stop=True,
                                )
                            r0 = ROW0 + di + (hf * HROWS if g == 0 else 0) + qi * (HROWS // 2)
                            vv = vt[:, r0 : r0 + HROWS // 2, COL0 + dj : COL0 + dj + W]
                            prod = tree_pool.tile(
                                [128, HROWS // 2, W], mmdt, tag="prod", bufs=4, name=f"cprod{g}"
                            )
                            nc.vector.tensor_tensor(
                                prod[:],
                                ab_ps.rearrange("p (a b) -> p a b", a=HROWS // 2),
                                vv,
                                AL.mult,
                            )
                            pf = prod.rearrange("p a b -> p (a b)")
                            # PE identity-accumulate into ACC
                            for n0, n1 in _bank_chunks(qi * (HALF // 2), (qi + 1) * (HALF // 2)):
                                bank = n0 // 512
                                st = j == 0 and bank not in started
                                if st:
                                    started.add(bank)
                                sp = j == KK - 1 and last_by_bank[bank] == (qi, n0)
                                nc.tensor.matmul(
                                    ACC_ps[:, n0:n1],
                                    ident[:],
                                    pf[:, n0 - qi * (HALF // 2) : n1 - qi * (HALF // 2)],
                                    start=st,
                                    stop=sp,
                                )
                    # transpose (PSUM -> SBUF) + output DMA for this half
                    t_sb = tree_pool.tile([128, HALF], fp32, tag="tout", bufs=2, name=f"t{g}{hf}")
                    nc.vector.transpose(t_sb[:], ACC_ps[:])
                    for bc in range(4):
                        src_ap = t_sb[bc * 32 : (bc + 1) * 32, :].rearrange(
                            "p (bp ci) -> p bp ci", ci=32
                        )
                        if g == 1:
                            c0 = 128 + (bc % 2) * 32
                            pxoff = (bc // 2) * (HALF // 32)
                            dst = o_view[:, pxoff : pxoff + HALF // 32, c0 : c0 + 32]
                        else:
                            c0 = bc * 32
                            pxoff = hf * (HALF // 32)
                            dst = o_view[:, pxoff : pxoff + HALF // 32, c0 : c0 + 32]
                        (nc.sync if bc % 2 == 0 else nc.scalar).dma_start(dst, src_ap)

            with tc.tile_pool(name="psC", bufs=1, space="PSUM") as psC_pool:
                ident = sel_sb["ident"]
                do_group(0, psC_pool, ident)
                do_group(1, psC_pool, ident)

    nc.compile()
    return nc, consts


_CACHE = {}


def _get_module():
    if "nc" not in _CACHE:
        _CACHE["nc"], _CACHE["consts"] = build_module()
    return _CACHE["nc"], _CACHE["consts"]


def make_in_maps(q, k, v, consts):
    in_maps = []
    for b in range(B):
        m = {
            "q": np.ascontiguousarray(q[b].reshape(C, H, W)),
            "k": np.ascontiguousarray(k[b].reshape(C, H, W)),
            "v": np.ascontiguousarray(v[b].reshape(C, H, W)),
        }
        m.update(consts)
        in_maps.append(m)
    return in_maps


def kernel(q: np.ndarray, k: np.ndarray, v: np.ndarray) -> np.ndarray:
    from concourse import bass_utils

    nc, consts = _get_module()
    in_maps = make_in_maps(np.asarray(q), np.asarray(k), np.asarray(v), consts)
    res = bass_utils.run_bass_kernel_spmd(nc, in_maps, core_ids=list(range(B)))
    out = np.stack([r["o"].reshape(H, W, C) for r in res.results])
    return out
